# revision 1
# baseline (speedup 1.0000x reference)
"""Trainium2 Bass kernel for nn_Attention_90125593739547.

Full-input contract: kernel(**inputs) takes the unsharded numpy inputs and
returns the full [S, B, D] output. Internally:
  - 8 NeuronCores, core c handles batch b = c // 4 and 4 heads (c % 4).
  - Per-core program (all matmuls fp32r, transposed-scores attention):
      xT = x.T done on HOST during sharding; DMA'd   [1024d, 2048t]
      qT = Wq.T @ xT + bq; kT = Wk.T @ xT + bk      [256hd, 2048]
      V~ = x @ [Wv | 0] + [bv | 1]                  [2048t, 4*65]  (ones col)
      per head pair, per q-half:
        St = kT_h.T-slice @ qT_h-slice (row-packed pairs)  [128t, 2*1024]
        pT = exp(SCALE * St)            (ScalarE, fp32r out)
        pv[65, 1024] += V~_h.T @ pT_h   (row 64 accumulates softmax denom)
        OT = pv[0:64] * recip(bcast(denom))   (K=1 ones matmul broadcast)
      y_partial = OT.T @ Wp_slice + bp_share        [2048, 1024]
  - Host sums the 4 per-head-group partials per batch (tensor-parallel reduce).
"""
import sys
sys.path.insert(0, '/opt/trn_rl_repo')
import numpy as np
from contextlib import ExitStack

S, B, D = 2048, 2, 1024
H, HD = 16, 64
SCALE = 1.0 / (HD ** 0.5)
P = 128
N_CORES = 8
CORES_PER_B = 4
NH = H // CORES_PER_B          # heads per core = 4
HDL = NH * HD                  # local head width = 256
SQ = S                         # q span per core (full sequence)
QH = 1024                      # q processed per attention stripe

_cache = {}


def _build():
    import concourse.bacc as bacc
    import concourse.mybir as mybir
    from concourse import tile

    F32 = mybir.dt.float32
    F32R = mybir.dt.float32r
    AF = mybir.ActivationFunctionType

    n_d, n_t, n_m = D // P, S // P, HDL // P
    n_qh = SQ // QH
    NV = NH * 65

    nc = bacc.Bacc("TRN2", target_bir_lowering=False, debug=False,
                   num_devices=N_CORES)

    x = nc.dram_tensor("x", [D, S], F32R, kind="ExternalInput")  # pre-transposed on host
    wq = nc.dram_tensor("wq", [D, HDL], F32R, kind="ExternalInput")
    wk = nc.dram_tensor("wk", [D, HDL], F32R, kind="ExternalInput")
    wv = nc.dram_tensor("wv", [D, NV], F32R, kind="ExternalInput")
    bq = nc.dram_tensor("bq", [1, HDL], F32R, kind="ExternalInput")
    bk = nc.dram_tensor("bk", [1, HDL], F32R, kind="ExternalInput")
    bv = nc.dram_tensor("bv", [1, NV], F32R, kind="ExternalInput")
    wp = nc.dram_tensor("wp", [HDL, D], F32R, kind="ExternalInput")
    bp = nc.dram_tensor("bp", [1, D], F32R, kind="ExternalInput")
    ones_d = nc.dram_tensor("ones", [1, 512], F32R, kind="ExternalInput")
    y = nc.dram_tensor("y", [SQ, D], F32, kind="ExternalOutput")

    with tile.TileContext(nc) as tc, ExitStack() as ctx:
        const = ctx.enter_context(tc.tile_pool(name="const", bufs=1))
        ones_row_t = const.tile([1, 512], F32R)
        nc.sync.dma_start(ones_row_t[:], ones_d[:, :])
        ones_row = ones_row_t[:]

        kv_pool = ctx.enter_context(tc.tile_pool(name="kv", bufs=1))
        qT = [kv_pool.tile([P, SQ], F32R, tag=f"qT{m}", name=f"qT{m}") for m in range(n_m)]
        kT = [kv_pool.tile([P, S], F32R, tag=f"kT{m}", name=f"kT{m}") for m in range(n_m)]
        Vt = [kv_pool.tile([P, NV], F32R, tag=f"V{t}", name=f"V{t}") for t in range(n_t)]
        OT = [kv_pool.tile([P, SQ], F32R, tag=f"OT{m}", name=f"OT{m}") for m in range(n_m)]

        proj_pool = ctx.enter_context(tc.tile_pool(name="proj", bufs=1))
        wp_sb = [proj_pool.tile([P, D], F32R, tag=f"wp{m}", name=f"wp{m}")
                 for m in range(n_m)]
        for m in range(n_m):
            nc.sync.dma_start(wp_sb[m][:], wp[m * P:(m + 1) * P, :])
        ystream = ctx.enter_context(tc.tile_pool(name="ystream", bufs=4))

        bias_pool = ctx.enter_context(tc.tile_pool(name="bias", bufs=1))
        bq_t = bias_pool.tile([1, HDL], F32R, tag="bq", name="bq")
        bk_t = bias_pool.tile([1, HDL], F32R, tag="bk", name="bk")
        bv_t = bias_pool.tile([1, NV], F32R, tag="bv", name="bv")
        bp_t = bias_pool.tile([1, D], F32R, tag="bp", name="bp")
        nc.sync.dma_start(bq_t[:], bq[:, :])
        nc.sync.dma_start(bk_t[:], bk[:, :])
        nc.sync.dma_start(bv_t[:], bv[:, :])
        nc.sync.dma_start(bp_t[:], bp[:, :])

        # ---- Phases A+B: transpose + QKV (xT/weights freed afterwards) ----
        with tc.tile_pool(name="xw", bufs=1) as xw_pool, \
             tc.tile_pool(name="psumAB", bufs=1, space="PSUM") as psum:
            xT = [xw_pool.tile([P, S], F32R, tag=f"xT{d}", name=f"xT{d}") for d in range(n_d)]
            wq_sb = [xw_pool.tile([P, HDL], F32R, tag=f"wq{d}", name=f"wq{d}") for d in range(n_d)]
            wk_sb = [xw_pool.tile([P, HDL], F32R, tag=f"wk{d}", name=f"wk{d}") for d in range(n_d)]
            wv_sb = [xw_pool.tile([P, NV], F32R, tag=f"wv{d}", name=f"wv{d}") for d in range(n_d)]
            # A: xT arrives pre-transposed from the host; straight DMA loads.
            # First halves land first so V/kq chains start after ~4MB.
            for half in (0, 1):
                for d in range(n_d):
                    nc.sync.dma_start(xT[d][:, half * 1024:(half + 1) * 1024],
                                      x[d * P:(d + 1) * P, half * 1024:(half + 1) * 1024])
                if half == 0:
                    for dd in range(n_d):
                        nc.sync.dma_start(wv_sb[dd][:], wv[dd * P:(dd + 1) * P, :])
                        nc.sync.dma_start(wk_sb[dd][:], wk[dd * P:(dd + 1) * P, :])
                        nc.sync.dma_start(wq_sb[dd][:], wq[dd * P:(dd + 1) * P, :])

            # B: V~ = x @ [Wv|0] + [bv|1]  (first: attention needs all of V)
            for tt in range(n_t):
                ps = psum.tile([P, NV], F32, tag="qkv2", name="qkv2", bufs=2)
                for lo in range(0, NV, 512):
                    w = min(512, NV - lo)
                    for dt in range(n_d):
                        nc.tensor.matmul(ps[:, lo:lo + w],
                                         xT[dt][:, tt * P:(tt + 1) * P],
                                         wv_sb[dt][:, lo:lo + w],
                                         start=(dt == 0), stop=False)
                    nc.tensor.matmul(ps[:, lo:lo + w], ones_row[0:1, 0:P],
                                     bv_t[0:1, lo:lo + w], start=False, stop=True)
                nc.vector.tensor_copy(Vt[tt][:], ps[:])

            # B: qT, kT (+bias via K=1 ones matmul), 512-wide chains
            kqi = 0
            for m in range(n_m):
                for dst, wsb, bias in ((kT, wk_sb, bk_t), (qT, wq_sb, bq_t)):
                    for lo in range(0, S, 512):
                        ps = psum.tile([P, 512], F32, tag="qkv", name="qkv", bufs=3)
                        for dt in range(n_d):
                            nc.tensor.matmul(
                                ps[:], wsb[dt][:, m * P:(m + 1) * P],
                                xT[dt][:, lo:lo + 512],
                                start=(dt == 0), stop=False)
                        nc.tensor.matmul(ps[:], bias[0:1, m * P:(m + 1) * P],
                                         ones_row[0:1, 0:512],
                                         start=False, stop=True)
                        if kqi % 2 == 0:
                            nc.vector.tensor_copy(dst[m][:, lo:lo + 512], ps[:])
                        else:
                            nc.scalar.copy(dst[m][:, lo:lo + 512], ps[:])
                        kqi += 1

        # ---- Phase C: attention ----
        # stripes: (q-block of 512) major, head-pair minor -> projection can
        # start on finished q-blocks while later stripes still run.
        QB = 512
        with tc.tile_pool(name="attn", bufs=2) as attn_pool, \
             tc.tile_pool(name="psumC", bufs=1, space="PSUM") as psum:
            for qb in range(SQ // QB):
                qlo = qb * QB
                for m in range(n_m):
                    pvA = psum.tile([65, QB], F32, tag="pvA", name="pvA", bufs=1)
                    pvB = psum.tile([65, QB], F32, tag="pvB", name="pvB", bufs=1)
                    for tt in range(n_t):
                        sc = psum.tile([P, 2 * QB], F32, tag="sc", name="sc", bufs=2)
                        for half, plo in ((0, 0), (1, 64)):
                            nc.tensor.matmul(
                                sc[:, half * QB: half * QB + QB],
                                kT[m][plo:plo + 64, tt * P:(tt + 1) * P],
                                qT[m][plo:plo + 64, qlo: qlo + QB],
                                start=True, stop=True)
                        pT = attn_pool.tile([P, 2 * QB], F32R, tag="pT", name="pT",
                                            bufs=4)
                        nc.scalar.activation(pT[:], sc[:], AF.Exp, scale=SCALE)
                        for half, pv in ((0, pvA), (1, pvB)):
                            h = 2 * m + half
                            nc.tensor.matmul(
                                pv[:], Vt[tt][:, h * 65:(h + 1) * 65],
                                pT[:, half * QB: half * QB + QB],
                                start=(tt == 0), stop=(tt == n_t - 1))
                    for half, pv in ((0, pvA), (1, pvB)):
                        plo = half * 64
                        den = attn_pool.tile([1, QB], F32, tag="den", name="den", bufs=3)
                        nc.vector.tensor_copy(den[:], pv[64:65, :])
                        ov = attn_pool.tile([64, QB], F32, tag="ov", name="ov", bufs=3)
                        nc.vector.tensor_copy(ov[:], pv[0:64, :])
                        dnb = attn_pool.tile([64, QB], F32, tag="dnb", name="dnb")
                        nc.gpsimd.partition_broadcast(dnb[:], den[0:1, :])
                        rcb = attn_pool.tile([64, QB], F32, tag="rcb", name="rcb")
                        nc.vector.reciprocal_approx_fast(rcb[:], dnb[:])
                        nc.vector.tensor_tensor(
                            OT[m][plo:plo + 64, qlo:qlo + QB],
                            ov[:], rcb[:], op=mybir.AluOpType.mult)

            # ---- Phase D: projection ----
            for qt in range(SQ // P):
                for nn in range(0, D, 512):
                    ps = psum.tile([P, 512], F32, tag="y", name="y", bufs=2)
                    for m in range(n_m):
                        nc.tensor.matmul(ps[:], OT[m][:, qt * P:(qt + 1) * P],
                                         wp_sb[m][:, nn:nn + 512],
                                         start=(m == 0), stop=False)
                    nc.tensor.matmul(ps[:], ones_row[0:1, 0:P], bp_t[0:1, nn:nn + 512],
                                     start=False, stop=True)
                    yt = ystream.tile([P, 512], F32, tag="y_out", name="y_out")
                    nc.vector.tensor_copy(yt[:], ps[:])
                    nc.sync.dma_start(y[qt * P:(qt + 1) * P, nn:nn + 512], yt[:])

    nc.compile()
    return nc


def _get_nc():
    if "nc" not in _cache:
        _cache["nc"] = _build()
    return _cache["nc"]


def make_in_maps(inputs, Wkv, bkv, Wq, bq, Wp, bp):
    """Host-side sharding: per-core input dicts."""
    inputs = np.asarray(inputs, dtype=np.float32)
    Wkv = np.asarray(Wkv, dtype=np.float32)
    bkv = np.asarray(bkv, dtype=np.float32)
    Wq = np.asarray(Wq, dtype=np.float32)
    bq = np.asarray(bq, dtype=np.float32)
    Wp = np.asarray(Wp, dtype=np.float32)
    bp = np.asarray(bp, dtype=np.float32)

    ones_np = np.ones((1, 512), dtype=np.float32)
    bp_np = bp.reshape(1, D)
    zeros_bp = np.zeros((1, D), dtype=np.float32)

    in_maps = []
    for c in range(N_CORES):
        b = c // CORES_PER_B
        g = c % CORES_PER_B
        hsl = slice(g * HDL, (g + 1) * HDL)
        x_b = np.ascontiguousarray(inputs[:, b, :].T)
        wq_c = np.ascontiguousarray(Wq[:, hsl])
        bq_c = np.ascontiguousarray(bq[hsl]).reshape(1, HDL)
        wk_c = np.ascontiguousarray(Wkv[:, hsl])
        bk_c = np.ascontiguousarray(bkv[hsl]).reshape(1, HDL)
        wv_full = Wkv[:, H * HD + g * HDL: H * HD + (g + 1) * HDL]
        bv_full = bkv[H * HD + g * HDL: H * HD + (g + 1) * HDL]
        wv_c = np.zeros((D, NH * 65), dtype=np.float32)
        bv_c = np.zeros((1, NH * 65), dtype=np.float32)
        for h in range(NH):
            wv_c[:, h * 65:h * 65 + 64] = wv_full[:, h * 64:(h + 1) * 64]
            bv_c[0, h * 65:h * 65 + 64] = bv_full[h * 64:(h + 1) * 64]
            bv_c[0, h * 65 + 64] = 1.0
        wp_c = np.ascontiguousarray(Wp[hsl, :])
        in_maps.append(dict(
            x=x_b, wq=wq_c, wk=wk_c, wv=wv_c, bq=bq_c, bk=bk_c, bv=bv_c,
            wp=wp_c, bp=(bp_np if g == 0 else zeros_bp),
            ones=ones_np))
    return in_maps


def combine_outputs(results):
    """Host-side unshard: sum head-group partials per batch."""
    out = np.zeros((S, B, D), dtype=np.float32)
    for b in range(B):
        acc = results[b * CORES_PER_B]["y"].copy()
        for g in range(1, CORES_PER_B):
            acc += results[b * CORES_PER_B + g]["y"]
        out[:, b, :] = acc
    return out


def kernel(inputs, Wkv, bkv, Wq, bq, Wp, bp):
    from concourse.bass_utils import run_bass_kernel_spmd
    nc = _get_nc()
    in_maps = make_in_maps(inputs, Wkv, bkv, Wq, bq, Wp, bp)
    res = run_bass_kernel_spmd(nc, in_maps, list(range(N_CORES)))
    return combine_outputs(res.results)



# revision 34
# speedup vs baseline: 1.2719x; 1.2719x over previous
"""Trainium2 Bass kernel for nn_Attention_90125593739547.

Full-input contract: kernel(**inputs) takes the unsharded numpy inputs and
returns the full [S, B, D] output. Internally:
  - 8 NeuronCores, core c handles batch b = c // 4 and 4 heads (c % 4).
  - All matmul operands are bf16 (fp32 PSUM accumulation); biases are folded
    into PSUM-evacuation vector ops; bp is added on host.
  - DRAM layouts are chunk-major so every DMA moves 4-8KB per partition row
    (descriptor-efficient).
  - Per-core program:
      phase A (per 512-seq chunk): kT = Wk.T@x (+bk), V~ = x@[Wv|0] (+[bv|1]),
      and for chunk 0 also qT; the first attention stripe (qb0, m0) is
      interleaved so the Activation engine starts exp early.
      phase B: remaining 7 stripes; qT for chunks 1-3 and the per-q-block
      projection are interleaved into the stripe stream (proj for block j is
      emitted after stripe (j+1, m0) so it never stalls the PE queue behind
      the DVE normalize chain).
      stripe (q-block 512, head-pair m):
        sc = kT_h.T-slice @ qT_h-slice per half   [128t, 2*512q] PSUM
        pT = exp(SCALE * sc)                       (ScalarE, bf16 out)
        pv[65, 1024] += V~_h.T @ pT_h              (row 64 = softmax denom)
        OT = pv[0:64] * bcast(recip(denom))        (DVE + gpsimd)
      proj: y = OT.T @ Wp (host adds bp + reduces 4 head-group partials).
"""
import sys
sys.path.insert(0, '/opt/trn_rl_repo')
import numpy as np
import ml_dtypes
from contextlib import ExitStack

S, B, D = 2048, 2, 1024
H, HD = 16, 64
SCALE = 1.0 / (HD ** 0.5)
P = 128
N_CORES = 8
CORES_PER_B = 4
NH = H // CORES_PER_B          # heads per core = 4
HDL = NH * HD                  # local head width = 256
NV = NH * 65                   # V~ width incl. denominator columns = 260
QB = 512                       # query block
CH = 512                       # seq chunk for QKV production

_cache = {}


def _build():
    import concourse.bacc as bacc
    import concourse.mybir as mybir
    from concourse import tile

    F32 = mybir.dt.float32
    BF16 = mybir.dt.bfloat16
    AF = mybir.ActivationFunctionType
    MUL = mybir.AluOpType.mult
    ADD = mybir.AluOpType.add

    n_d, n_t, n_m = D // P, S // P, HDL // P     # 8, 16, 2
    n_qb = S // QB                               # 4
    n_ch = S // CH                               # 4
    tpc = CH // P                                # seq tiles per chunk = 4
    XW = n_d * CH                                # x chunk width = 4096

    nc = bacc.Bacc("TRN2", target_bir_lowering=False, debug=False,
                   num_devices=N_CORES)

    # Chunk-major layouts (see make_in_maps for the host-side reshapes).
    x = nc.dram_tensor("x", [n_ch * P, XW], BF16, kind="ExternalInput")
    wq = nc.dram_tensor("wq", [P, n_d * HDL], BF16, kind="ExternalInput")
    wk = nc.dram_tensor("wk", [P, n_d * HDL], BF16, kind="ExternalInput")
    wv = nc.dram_tensor("wv", [P, n_d * NV], BF16, kind="ExternalInput")
    bq = nc.dram_tensor("bq", [P, n_m], F32, kind="ExternalInput")
    bk = nc.dram_tensor("bk", [P, n_m], F32, kind="ExternalInput")
    bv = nc.dram_tensor("bv", [1, NV], F32, kind="ExternalInput")
    wp = nc.dram_tensor("wp", [P, n_m * D], BF16, kind="ExternalInput")
    y = nc.dram_tensor("y", [S, D], F32, kind="ExternalOutput")
    import os as _os

    with tile.TileContext(nc) as tc, ExitStack() as ctx:
        sb = ctx.enter_context(tc.tile_pool(name="sb", bufs=1))
        xb = sb.tile([P, n_ch * XW], BF16, tag="xb", name="xb")
        wq_sb = sb.tile([P, n_d * HDL], BF16, tag="wq", name="wq")
        wk_sb = sb.tile([P, n_d * HDL], BF16, tag="wk", name="wk")
        wv_sb = sb.tile([P, n_d * NV], BF16, tag="wv", name="wv")
        wp_sb = sb.tile([P, n_m * D], BF16, tag="wp", name="wp")
        qT = [sb.tile([P, S], BF16, tag=f"qT{m}", name=f"qT{m}") for m in range(n_m)]
        kT = [sb.tile([P, S], BF16, tag=f"kT{m}", name=f"kT{m}") for m in range(n_m)]
        Vt = [sb.tile([P, NV], BF16, tag=f"V{t}", name=f"V{t}") for t in range(n_t)]
        OT = [sb.tile([P, S], BF16, tag=f"OT{m}", name=f"OT{m}") for m in range(n_m)]
        bq_sb = sb.tile([P, n_m], F32, tag="bq", name="bq")
        bk_sb = sb.tile([P, n_m], F32, tag="bk", name="bk")
        bv_row = sb.tile([1, NV], F32, tag="bvr", name="bvr")
        bvb = sb.tile([P, NV], F32, tag="bvb", name="bvb")

        def xsl(c, d, off, w):
            return xb[:, c * XW + d * CH + off: c * XW + d * CH + off + w]

        pT_pool = ctx.enter_context(tc.tile_pool(name="pT", bufs=4))
        nrm = ctx.enter_context(tc.tile_pool(name="nrm", bufs=2))
        ystream = ctx.enter_context(tc.tile_pool(name="ystream", bufs=3))

        # Persistent PSUM: sc (2 banks x bufs=2) + pv (2 banks) = 6 banks.
        psA = ctx.enter_context(tc.tile_pool(name="psA", bufs=1, space="PSUM"))

        # HWLOOP=N wraps the whole body in a hardware loop for on-device
        # timing: per-iter time = (wall(N) - wall(1)) / (N - 1).
        HWLOOP = int(_os.environ.get("HWLOOP", "0"))
        if HWLOOP:
            ctx.enter_context(tc.For_i(0, HWLOOP))

        # ---- DMA queue: wk/chunk-0 halves first so PE starts ~2.5us in ----
        HK = n_d * HDL // 2
        nc.sync.dma_start(wk_sb[:, 0:HK], wk[:, 0:HK])
        nc.sync.dma_start(xb[:, 0:XW // 2], x[0:P, 0:XW // 2])
        nc.sync.dma_start(wk_sb[:, HK:2 * HK], wk[:, HK:2 * HK])
        nc.sync.dma_start(xb[:, XW // 2:XW], x[0:P, XW // 2:XW])
        nc.sync.dma_start(bq_sb[:], bq[:, :])
        nc.sync.dma_start(bk_sb[:], bk[:, :])
        nc.sync.dma_start(bv_row[:], bv[:, :])
        nc.gpsimd.partition_broadcast(bvb[:], bv_row[0:1, :])
        nc.sync.dma_start(wv_sb[:], wv[:, :])
        nc.sync.dma_start(wq_sb[:], wq[:, :])
        for c in range(1, n_ch):
            nc.sync.dma_start(xb[:, c * XW:(c + 1) * XW], x[c * P:(c + 1) * P, :])
        nc.sync.dma_start(wp_sb[:], wp[:, :])

        # sc and pT are [P, 2, QB]: the two heads' halves anchor at fixed
        # half*QB offsets so matmul outputs always start at a PSUM bank
        # boundary (mid-bank matmul outputs crash the runtime), and the exp
        # reads both halves in one (possibly strided) access pattern.
        def emit_sc(qlo, qw, m, tt):
            """Scores for one (query-range, head-pair, key-tile) -> PSUM tile."""
            sc = psA.tile([P, 2, QB], F32, tag="sc", name="sc", bufs=2)
            for half in (0, 1):
                plo = half * 64
                nc.tensor.matmul(
                    sc[:, half, 0:qw],
                    kT[m][plo:plo + 64, tt * P:(tt + 1) * P],
                    qT[m][plo:plo + 64, qlo:qlo + qw],
                    start=True, stop=True)
            return sc

        def emit_exp(sc, qw):
            pT = pT_pool.tile([P, 2, QB], BF16, tag="pT", name="pT", bufs=4)
            nc.scalar.activation(pT[:, :, 0:qw], sc[:, :, 0:qw],
                                 AF.Exp, scale=SCALE)
            return pT

        def emit_pv(qw, m, tt, pv, pT):
            for half in (0, 1):
                h = 2 * m + half
                nc.tensor.matmul(
                    pv[0:65, half * QB: half * QB + qw],
                    Vt[tt][:, h * 65:(h + 1) * 65],
                    pT[:, half, 0:qw],
                    start=(tt == 0), stop=(tt == n_t - 1))

        def emit_stripe_iter(qlo, qw, m, tt, pv):
            sc = emit_sc(qlo, qw, m, tt)
            pT = emit_exp(sc, qw)
            emit_pv(qw, m, tt, pv, pT)

        def emit_normalize(qlo, qw, m, pv):
            """OT[m][:, query-range] = pv[0:64] * bcast(1/pv[64]).

            pv is first copied to SBUF in one DVE op so the PSUM banks free
            ~1.2us after the last pv matmul instead of after the full
            recip->broadcast->mult chain (kills the stripe-boundary stall)."""
            # single copy frees the pv PSUM banks ~1.2us after the last pv
            # matmul; everything below reads the SBUF copy.
            pvS = nrm.tile([65, 2 * QB], F32, tag="pvS", name="pvS", bufs=2)
            nc.vector.tensor_copy(pvS[:], pv[:])
            dens = []
            for half in (0, 1):
                co = half * QB
                # denominator row to a partition-0 tile (gpsimd broadcast and
                # the custom-DVE reciprocal both need partition-0 sources)
                den = nrm.tile([1, QB], F32, tag=f"den{half}",
                               name=f"den{half}", bufs=2)
                nc.vector.tensor_copy(den[0:1, 0:qw], pvS[64:65, co:co + qw])
                dens.append(den)
            for half in (0, 1):
                plo = half * 64
                co = half * QB
                rb = nrm.tile([64, QB], F32, tag="rb", name="rb", bufs=2)
                nc.gpsimd.partition_broadcast(rb[0:64, 0:qw],
                                              dens[half][0:1, 0:qw])
                rc = nrm.tile([64, QB], F32, tag="rc", name="rc", bufs=2)
                nc.vector.reciprocal_approx_fast(rc[0:64, 0:qw], rb[0:64, 0:qw])
                nc.vector.tensor_tensor(
                    OT[m][plo:plo + 64, qlo:qlo + qw],
                    pvS[0:64, co:co + qw], rc[0:64, 0:qw], op=MUL)

        def emit_qkT(c, m, dst_list, w_big, b_sb, psum_pool, ptag, pbufs):
            """dst[m][:, chunk c] = W.T @ x + b (per-partition bias)."""
            clo = c * CH
            ps = psum_pool.tile([P, CH], F32, tag=ptag, name=ptag, bufs=pbufs)
            for dt in range(n_d):
                nc.tensor.matmul(ps[:], w_big[:, dt * HDL + m * P: dt * HDL + (m + 1) * P],
                                 xsl(c, dt, 0, CH),
                                 start=(dt == 0), stop=(dt == n_d - 1))
            nc.vector.tensor_scalar_add(out=dst_list[m][:, clo:clo + CH],
                                        in0=ps[:], scalar1=b_sb[:, m:m + 1])

        # ---- Phase A: kT/V~ chunk-by-chunk, qT chunk 0, stripe (qb0, m0) ----
        pv0 = psA.tile([65, 2 * QB], F32, tag="pv", name="pv", bufs=1)
        with tc.tile_pool(name="psB", bufs=1, space="PSUM") as psB:
            for c in range(n_ch):
                for m in range(n_m):
                    emit_qkT(c, m, kT, wk_sb, bk_sb, psB, "qkv", 2)
                for t in range(tpc):
                    tt = c * tpc + t
                    ps = psB.tile([P, NV], F32, tag="qkv", name="qkv", bufs=2)
                    for dt in range(n_d):
                        nc.tensor.matmul(ps[:], xsl(c, dt, t * P, P),
                                         wv_sb[:, dt * NV:(dt + 1) * NV],
                                         start=(dt == 0), stop=(dt == n_d - 1))
                    nc.vector.tensor_tensor(Vt[tt][:], ps[:], bvb[:], op=ADD)
                if c == 0:
                    for m in range(n_m):
                        emit_qkT(0, m, qT, wq_sb, bq_sb, psB, "qkv", 2)
                # interleave stripe (qb0, m0) for the key tiles just produced
                for t in range(tpc):
                    emit_stripe_iter(0, QB, 0, c * tpc + t, pv0)
        emit_normalize(0, QB, 0, pv0)

        # ---- Phase B: remaining stripes; qT chunks 1-3 and proj interleaved
        # as fine-grained PE fillers (one matmul per stripe iteration) so the
        # Activation engine never starves while a filler chain runs. ----
        with tc.tile_pool(name="psC", bufs=1, space="PSUM") as psC:
            def gen_qkT_filler(c, m):
                """Yield per-matmul steps of qT[m][:, chunk c] production."""
                clo = c * CH
                ps = psC.tile([P, CH], F32, tag="y", name="y", bufs=2)
                for dt in range(n_d):
                    nc.tensor.matmul(
                        ps[:], wq_sb[:, dt * HDL + m * P: dt * HDL + (m + 1) * P],
                        xsl(c, dt, 0, CH),
                        start=(dt == 0), stop=(dt == n_d - 1))
                    if dt < n_d - 1:
                        yield
                nc.vector.tensor_scalar_add(out=qT[m][:, clo:clo + CH],
                                            in0=ps[:], scalar1=bq_sb[:, m:m + 1])
                yield

            def gen_proj_filler(qlo, qw):
                """Yield per-matmul steps of the [qlo, qlo+qw) projection."""
                for qt in range(qw // P):
                    rlo = qlo + qt * P
                    yt = ystream.tile([P, D], F32, tag="yt", name="yt", bufs=3)
                    for nn in (0, 512):
                        yp = psC.tile([P, 512], F32, tag="y", name="y", bufs=2)
                        for m in range(n_m):
                            nc.tensor.matmul(yp[:], OT[m][:, rlo:rlo + P],
                                             wp_sb[:, m * D + nn: m * D + nn + 512],
                                             start=(m == 0), stop=(m == n_m - 1))
                            if m < n_m - 1:
                                yield
                        nc.vector.tensor_copy(yt[:, nn:nn + 512], yp[:])
                        yield
                    nc.sync.dma_start(y[rlo:rlo + P, :], yt[:])

            def chain(*gens):
                for g in gens:
                    yield from g

            def drain(filler):
                if filler is not None:
                    for _ in filler:
                        pass

            # (stripe, filler, start_at) schedule: qT chunk c lands during the
            # stripe before q-block c; proj(qb) lands during the stripe after
            # (qb, m1), delayed 6 iters so its first matmul never waits on the
            # normalize chain. The last q-block's m1 stripe is split into two
            # 256-query half-stripes so the final normalize+projection overlap
            # the second half's compute and the tail shrinks to ~5us.
            sched = [
                ((0, QB, 1), chain(gen_qkT_filler(1, 0), gen_qkT_filler(1, 1)), 0),
                ((QB, QB, 0), gen_proj_filler(0, QB), 6),
                ((QB, QB, 1), chain(gen_qkT_filler(2, 0), gen_qkT_filler(2, 1)), 0),
                ((2 * QB, QB, 0), gen_proj_filler(QB, QB), 6),
                ((2 * QB, QB, 1), chain(gen_qkT_filler(3, 0), gen_qkT_filler(3, 1)), 0),
                ((3 * QB, QB, 0), gen_proj_filler(2 * QB, QB), 6),
                ((3 * QB, QB // 2, 1), None, 0),
                ((3 * QB + QB // 2, QB // 2, 1),
                 gen_proj_filler(3 * QB, QB // 2), 2),
            ]
            # Software-pipelined flat loop: the NEXT iteration's scores are
            # emitted on PE before this iteration's pv, so the PE computes
            # sc(i+1) during exp(i) and the exp->pv->sc->exp latency chain
            # never gates the Activation engine.
            flat = []
            for (qlo, qw, m), filler, start_at in sched:
                for tt in range(n_t):
                    flat.append((qlo, qw, m, tt, filler, start_at))
            pv_tiles = {}
            sc_cur = emit_sc(*flat[0][:4]) if flat else None
            for gi, (qlo, qw, m, tt, filler, start_at) in enumerate(flat):
                pT = emit_exp(sc_cur, qw)
                if gi + 1 < len(flat):
                    sc_cur = emit_sc(*flat[gi + 1][:4])
                key = (qlo, m)
                if key not in pv_tiles:
                    pv_tiles[key] = psA.tile([65, 2 * QB], F32, tag="pv",
                                             name="pv", bufs=1)
                emit_pv(qw, m, tt, pv_tiles[key], pT)
                if filler is not None and tt >= start_at:
                    n_fill = 16 - start_at
                    steps = (16 * (tt - start_at + 1) + n_fill - 1) // n_fill \
                        - (16 * (tt - start_at) + n_fill - 1) // n_fill
                    for _ in range(max(1, steps)):
                        next(filler, None)
                if tt == n_t - 1:
                    emit_normalize(qlo, qw, m, pv_tiles[key])
                    drain(filler)
            drain(gen_proj_filler(3 * QB + QB // 2, QB // 2))

    nc.compile()
    return nc


def _get_nc():
    if "nc" not in _cache:
        _cache["nc"] = _build()
    return _cache["nc"]


def make_in_maps(inputs, Wkv, bkv, Wq, bq, Wp, bp):
    """Host-side sharding: per-core input dicts (bf16, chunk-major layouts)."""
    BF = ml_dtypes.bfloat16
    n_d, n_m, n_ch = D // P, HDL // P, S // CH
    inputs = np.asarray(inputs, dtype=np.float32)
    Wkv = np.asarray(Wkv, dtype=np.float32)
    bkv = np.asarray(bkv, dtype=np.float32)
    Wq = np.asarray(Wq, dtype=np.float32)
    bq = np.asarray(bq, dtype=np.float32)
    Wp = np.asarray(Wp, dtype=np.float32)

    def dmaj(w, width):
        # [D, width] -> [128, n_d*width] rows: row p = [d0 | d1 | ...]
        return np.ascontiguousarray(
            w.reshape(n_d, P, width).transpose(1, 0, 2).reshape(P, n_d * width))

    in_maps = []
    for c in range(N_CORES):
        b = c // CORES_PER_B
        g = c % CORES_PER_B
        hsl = slice(g * HDL, (g + 1) * HDL)
        X = inputs[:, b, :].T                      # [D, S]
        # [d, p, ch, j] -> [ch, p, d, j] -> [n_ch*128, n_d*CH]
        x_c = np.ascontiguousarray(
            X.reshape(n_d, P, n_ch, CH).transpose(2, 1, 0, 3)
            .reshape(n_ch * P, n_d * CH)).astype(BF)
        wq_c = dmaj(Wq[:, hsl], HDL).astype(BF)
        wk_c = dmaj(Wkv[:, hsl], HDL).astype(BF)
        bq_c = np.ascontiguousarray(bq[hsl].reshape(n_m, P).T)
        bk_c = np.ascontiguousarray(bkv[hsl].reshape(n_m, P).T)
        wv_full = Wkv[:, H * HD + g * HDL: H * HD + (g + 1) * HDL]
        bv_full = bkv[H * HD + g * HDL: H * HD + (g + 1) * HDL]
        wv_c = np.zeros((D, NV), dtype=np.float32)
        bv_c = np.zeros((1, NV), dtype=np.float32)
        for h in range(NH):
            wv_c[:, h * 65:h * 65 + 64] = wv_full[:, h * 64:(h + 1) * 64]
            bv_c[0, h * 65:h * 65 + 64] = bv_full[h * 64:(h + 1) * 64]
            bv_c[0, h * 65 + 64] = 1.0
        wv_c = dmaj(wv_c, NV).astype(BF)
        # Wp [HDL, D] -> [128, n_m*D]
        wp_c = np.ascontiguousarray(
            Wp[hsl, :].reshape(n_m, P, D).transpose(1, 0, 2).reshape(P, n_m * D)
        ).astype(BF)
        in_maps.append(dict(
            x=x_c, wq=wq_c, wk=wk_c, wv=wv_c,
            bq=bq_c, bk=bk_c, bv=bv_c, wp=wp_c))
    return in_maps


def combine_outputs(results, bp):
    """Host-side unshard: sum head-group partials per batch, add bp."""
    bp = np.asarray(bp, dtype=np.float32)
    out = np.zeros((S, B, D), dtype=np.float32)
    for b in range(B):
        acc = results[b * CORES_PER_B]["y"].copy()
        for g in range(1, CORES_PER_B):
            acc += results[b * CORES_PER_B + g]["y"]
        out[:, b, :] = acc + bp
    return out


def kernel(inputs, Wkv, bkv, Wq, bq, Wp, bp):
    from concourse.bass_utils import run_bass_kernel_spmd
    nc = _get_nc()
    in_maps = make_in_maps(inputs, Wkv, bkv, Wq, bq, Wp, bp)
    res = run_bass_kernel_spmd(nc, in_maps, list(range(N_CORES)))
    return combine_outputs(res.results, bp)
